# revision 1
# baseline (speedup 1.0000x reference)
"""Multi-head causal attention (B=2,S=2048,D=1024,H=16,RoPE) on 8 TRN2 NeuronCores.

Sharding: core c handles batch b=c//4, head-group g=c%4 (4 heads each).
Wq/Wk/Wv column-sharded per head group, Wo row-sharded; the all-reduce over
head groups is realized as a host-side partial sum at gather time.

Per-core kernel (all matmuls in fp32r = fp32 with 11-bit mantissa, full PE rate):
  Phase A: QKV projections from pre-transposed x (feature-major), Q/K kept
           feature-major [d, s] and RoPE'd in place; V natural [s, d] with a
           ones column appended per head (softmax denominators ride the AV
           matmul for free).
  Phase B: per head: scoresT[k,q] = K^T-major matmul (no transposes anywhere),
           causal block skipping, exp on ACT straight out of PSUM, binary
           diag-mask multiply, AV accumulation into PSUM [65, 512].
  Phase C: normalize by broadcasted reciprocal row sums, output projection,
           partial out [2048, 1024] -> DRAM.

RoPE trick: rotation pairs (j, j+32) are interleaved adjacently on partitions
(p ^ 1), so the partition shift is two stride-2 SBUF->SBUF DMAs; sin table has
the sign pattern baked in host-side.
"""
import numpy as np
from contextlib import ExitStack

import concourse.bass as bass
import concourse.tile as tile
from concourse import mybir
from concourse.bass_utils import run_bass_kernel_spmd

B, S, D, H, HD = 2, 2048, 1024, 16, 64
HPC = 4            # heads per core
DC = HPC * HD      # 256 features per core
NDT = D // 128     # 8 input-dim tiles
NST = S // 128     # 16 sequence/key tiles
NQB = S // 512     # 4 query blocks
MT = DC // 128     # 2 feature m-tiles for Q/K/vec

F32 = mybir.dt.float32
F32R = mybir.dt.float32r
AF = mybir.ActivationFunctionType

_nop_ctr = [0]


def fix_engine_waits(nc, max_waits=1):
    """This walrus build rejects any engine instruction with >1 sync wait
    (single wait slot per instruction struct). Move excess waits onto
    same-engine NoOps inserted just before, one wait per NoOp. InstISA is
    skipped (fixed-length encoding)."""
    moved = 0
    for f in nc.m.functions:
        for b in f.blocks:
            insts = b.instructions
            i = 0
            while i < len(insts):
                inst = insts[i]
                if inst.opcode != "ISA" and inst.sync_info is not None:
                    si = inst.sync_info
                    waits = list(si.on_wait)
                    if len(waits) > max_waits:
                        keep = waits[-max_waits:]
                        for w in waits[:-max_waits]:
                            _nop_ctr[0] += 1
                            moved += 1
                            nop = mybir.InstNoOp(
                                name=f"I-waitnop-{_nop_ctr[0]}", ins=[], outs=[]
                            )
                            nop.engine = inst.engine
                            nop.sync_info = mybir.SyncInfo(on_wait=[w], on_update=[])
                            insts.insert(i, nop)
                            i += 1
                        si.on_wait = keep
                        inst.sync_info = si
                i += 1
    return moved


def _attention_body(ctx: ExitStack, tc, inp, out_ap):
    nc = tc.nc

    persist = ctx.enter_context(tc.tile_pool(name="persist", bufs=1))
    exp_p = ctx.enter_context(tc.tile_pool(name="expp", bufs=8))
    rrow_p = ctx.enter_context(tc.tile_pool(name="rrow", bufs=2))
    rec_p = ctx.enter_context(tc.tile_pool(name="recp", bufs=2))
    vtmp_p = ctx.enter_context(tc.tile_pool(name="vtmp", bufs=2))
    tout_p = ctx.enter_context(tc.tile_pool(name="toutp", bufs=2))
    ps_work = ctx.enter_context(tc.tile_pool(name="ps_work", bufs=5, space="PSUM"))
    ps_uvec = ctx.enter_context(tc.tile_pool(name="ps_uvec", bufs=3, space="PSUM"))
    wpool = ctx.enter_context(tc.tile_pool(name="wpool", bufs=1))
    xtp = ctx.enter_context(tc.tile_pool(name="xtp", bufs=2))
    qtmp_p = ctx.enter_context(tc.tile_pool(name="qtmp", bufs=3))
    tsh_p = ctx.enter_context(tc.tile_pool(name="tsh", bufs=3))
    tb2_p = ctx.enter_context(tc.tile_pool(name="tb2", bufs=2))

    # ---- persistent tensors ----
    qrt = persist.tile([128, MT, S], F32R)      # rotated Q^T  (d-major)
    krt = persist.tile([128, MT, S], F32R)      # rotated K^T
    vext = persist.tile([128, NST, HPC * 65], F32R)  # V tiles + ones col per head
    vecT = persist.tile([128, MT, S], F32R)     # normalized attention output^T
    cos_sb = persist.tile([128, S], F32)
    sin_sb = persist.tile([128, S], F32)
    wo_sb = persist.tile([128, MT, D], F32R)
    mdiag_sb = persist.tile([128, 128], F32R)   # binary causal mask, diag block^T
    bq_sb = persist.tile([128, MT], F32)
    bv_sb = persist.tile([1, DC], F32R)
    ones_sb = persist.tile([1, 128], F32R)

    # weights first (per d-tile so the first matmuls start early), then consts
    wq_sb = wpool.tile([128, NDT, DC], F32R)
    wk_sb = wpool.tile([128, NDT, DC], F32R)
    wv_sb = wpool.tile([128, NDT, DC], F32R)
    xT_view = inp["xT"].rearrange("(dt p) s -> p dt s", p=128)
    xt0 = xtp.tile([128, NDT, 512], F32R, tag="xt", name="xt0")
    for dt in range(NDT):
        nc.sync.dma_start(xt0[:, dt, :], xT_view[:, dt, 0:512])
        for w_sb, nm in ((wq_sb, "wqT"), (wk_sb, "wkT"), (wv_sb, "wvT")):
            nc.sync.dma_start(
                w_sb[:, dt, :],
                inp[nm].rearrange("(dt p) o -> p dt o", p=128)[:, dt, :],
            )
    nc.sync.dma_start(cos_sb[:, :], inp["cosT"])
    nc.sync.dma_start(sin_sb[:, :], inp["sinT"])
    nc.sync.dma_start(mdiag_sb[:, :], inp["mdiagT"])
    nc.sync.dma_start(bq_sb[:, :], inp["bqc"])
    nc.sync.dma_start(bv_sb[:, :], inp["bvr"])
    nc.sync.dma_start(ones_sb[:, :], inp["ones"])
    nc.sync.dma_start(
        wo_sb[:, :, :], inp["woT"].rearrange("(mt p) o -> p mt o", p=128)
    )
    # ones columns of vext (col 64 of each head slot, every k-tile)
    vones_dst = vext[:, :, :].rearrange("p st (h e) -> p st h e", e=65)[:, :, :, 64:65]
    nc.sync.dma_start(vones_dst, inp["vones"].rearrange("p (st h e) -> p st h e", st=NST, h=HPC))

    # ---- Phases A/B interleaved ----
    # For each head-pair m: project+RoPE Q/K (m only), then run attention for
    # m; V-projection s-tiles are emitted just-in-time per q-block (m=0) so
    # PE work fills the ACT-bound attention stretches. Sub-range deps let
    # attention on q-block qb start as soon as projections for s-block qb land.
    def qk_proj_sb(m, sb, xt, which):
            ssl = slice(sb * 512, (sb + 1) * 512)
            for w_sb, dst, is_q in (((wq_sb, qrt, True),) if which == 0 else ((wk_sb, krt, False),)):
                tag_q = "q" if is_q else "k"
                psq = ps_work.tile([128, 512], F32, tag="w", name=f"psq{tag_q}_{m}_{sb}")
                for dt in range(NDT):
                    nc.tensor.matmul(
                        psq[:, :],
                        w_sb[:, dt, m * 128 : (m + 1) * 128],
                        xt[:, dt, :],
                        start=(dt == 0),
                        stop=(dt == NDT - 1),
                    )
                qt = qtmp_p.tile([128, 512], F32, tag="qt", name=f"qt{tag_q}_{m}_{sb}")
                if is_q:
                    nc.scalar.activation(
                        qt[:, :], psq[:, :], AF.Identity, bias=bq_sb[:, m : m + 1]
                    )
                else:
                    nc.scalar.copy(qt[:, :], psq[:, :])
                # rotate_half partition shift p ^ 32 via 4 contiguous DMAs
                sh = tsh_p.tile([128, 512], F32, tag="sh", name=f"sh{tag_q}_{m}_{sb}")
                for base in (0, 64):
                    nc.sync.dma_start(
                        sh[base : base + 32, :], qt[base + 32 : base + 64, :]
                    )
                    nc.sync.dma_start(
                        sh[base + 32 : base + 64, :], qt[base : base + 32, :]
                    )
                dsl = dst[:, m, ssl]
                tb2 = tb2_p.tile([128, 512], F32, tag="tb2", name=f"tb2{tag_q}_{m}_{sb}")
                nc.vector.tensor_mul(dsl, qt[:, :], cos_sb[:, ssl])
                nc.vector.tensor_mul(tb2[:, :], sh[:, :], sin_sb[:, ssl])
                nc.vector.tensor_add(dsl, dsl, tb2[:, :])

    def v_proj(st, xt):
        psv = ps_work.tile([128, 512], F32, tag="w", name=f"psv_{st}")[:, 0:256]
        for dt in range(NDT):
            nc.tensor.matmul(
                psv[:, :],
                xt[:, dt, (st % 4) * 128 : (st % 4 + 1) * 128],
                wv_sb[:, dt, :],
                start=(dt == 0),
                stop=False,
            )
        nc.tensor.matmul(
            psv[:, :], ones_sb[0:1, :], bv_sb[0:1, :], start=False, stop=True
        )
        vdst = vext[:, st, :].rearrange("p (h e) -> p h e", e=65)[:, :, 0:64]
        nc.scalar.copy(vdst, psv[:, :].rearrange("p (h e) -> p h e", e=64))

    def load_xts(pref):
        xts = []
        for sb in range(NQB):
            xtn = xtp.tile([128, NDT, 512], F32R, tag="xt", name=f"xt{pref}_{sb}")
            for dt in range(NDT):
                nc.sync.dma_start(
                    xtn[:, dt, :], xT_view[:, dt, sb * 512 : (sb + 1) * 512]
                )
            xts.append(xtn)
        return xts

    def normalize(h, qb, puv_t):
        m, pb = h // 2, (h % 2) * 64
        ridx = h * NQB + qb
        srow = rrow_p.tile([1, 512], F32, tag="rr", name=f"srow_{h}_{qb}")
        nc.vector.tensor_copy(srow[:, :], puv_t[64:65, :])
        nc.sync.dma_start(inp["recs"][ridx : ridx + 1, :], srow[0:1, :])
        # spread the 512 sums over 32 partitions so reciprocal runs parallel
        sspread = inp["recs"][ridx : ridx + 1, :].rearrange(
            "one (p j) -> (one p) j", j=16
        )
        rspread = inp["recr"][ridx : ridx + 1, :].rearrange(
            "one (p j) -> (one p) j", j=16
        )
        scol = rec_p.tile([32, 16], F32, tag="sc", name=f"scol_{h}_{qb}")
        nc.sync.dma_start(scol[:, :], sspread)
        nc.vector.reciprocal(scol[:, :], scol[:, :])
        nc.sync.dma_start(rspread, scol[:, :])
        rec = rec_p.tile([64, 512], F32, tag="rc", name=f"rec_{h}_{qb}")
        nc.sync.dma_start(
            rec[:, :], inp["recr"][ridx : ridx + 1, :].to_broadcast([64, 512])
        )
        qsl = slice(qb * 512, (qb + 1) * 512)
        if pb == 0:
            nc.vector.tensor_mul(vecT[0:64, m, qsl], puv_t[0:64, :], rec[:, :])
        else:
            vt = vtmp_p.tile([64, 512], F32R, tag="vt", name=f"vt_{h}_{qb}")
            nc.vector.tensor_mul(vt[:, :], puv_t[0:64, :], rec[:, :])
            nc.sync.dma_start(vecT[64:128, m, qsl], vt[:, :])

    xts = [xt0]
    for sb in range(1, NQB):
        xtn = xtp.tile([128, NDT, 512], F32R, tag="xt", name=f"xt0_{sb}")
        for dt in range(NDT):
            nc.sync.dma_start(
                xtn[:, dt, :], xT_view[:, dt, sb * 512 : (sb + 1) * 512]
            )
        xts.append(xtn)
    def attn_qb(m, qb):
        puv = [
            ps_uvec.tile([65, 512], F32, tag="u", name=f"puv_m{m}h{hp}q{qb}")
            for hp in range(2)
        ]
        pending = None
        for kt in range(4 * qb + 5):
            if kt <= 4 * qb + 3:
                qb0 = kt // 4
                c0 = (kt % 4) * 128 if qb == qb0 else 0
                ets = []
                for hp in range(2):
                    pb = hp * 64
                    psc = ps_work.tile([128, 512], F32, tag="w", name=f"psc{hp}")
                    nc.tensor.matmul(
                        psc[:, c0:512],
                        krt[pb : pb + 64, m, kt * 128 : (kt + 1) * 128],
                        qrt[pb : pb + 64, m, qb * 512 + c0 : (qb + 1) * 512],
                        start=True,
                        stop=True,
                    )
                    ets.append(
                        (psc, exp_p.tile([128, 512], F32R, tag="e", name=f"et{hp}"))
                    )
                for hp, (psc, et) in enumerate(ets):
                    nc.scalar.activation(
                        et[:, c0:512], psc[:, c0:512], AF.Exp, scale=0.125
                    )
                    if qb == qb0:
                        nc.gpsimd.tensor_mul(
                            et[:, c0 : c0 + 128],
                            et[:, c0 : c0 + 128],
                            mdiag_sb[:, :],
                        )
            else:
                ets = None
            if pending is not None:
                pkt, pc0, pets = pending
                for hp, (psc, et) in enumerate(pets):
                    h = 2 * m + hp
                    nc.tensor.matmul(
                        puv[hp][:, pc0:512],
                        vext[:, pkt, h * 65 : (h + 1) * 65],
                        et[:, pc0:512],
                        start=(pkt == 0),
                        stop=(pkt == qb * 4 + 3),
                        skip_group_check=True,
                    )
            if ets is not None:
                pending = (kt, c0, ets)
        for hp in range(2):
            normalize(2 * m + hp, qb, puv[hp])

    for sb in range(NQB):
        for m in range(MT):
            qk_proj_sb(m, sb, xts[sb], 0)
        v_proj(4 * sb, xts[sb])
        v_proj(4 * sb + 1, xts[sb])
        for m in range(MT):
            qk_proj_sb(m, sb, xts[sb], 1)
        v_proj(4 * sb + 2, xts[sb])
        v_proj(4 * sb + 3, xts[sb])
    for m in range(MT):
        for qb in range(NQB):
            attn_qb(m, qb)

    # ---- Phase C: output projection ----
    for qt_i in range(NST):
        qsl = slice(qt_i * 128, (qt_i + 1) * 128)
        for oc in range(2):
            osl = slice(oc * 512, (oc + 1) * 512)
            pso = ps_work.tile([128, 512], F32, tag="w", name="pso")
            for mt in range(MT):
                nc.tensor.matmul(
                    pso[:, :],
                    vecT[:, mt, qsl],
                    wo_sb[:, mt, osl],
                    start=(mt == 0),
                    stop=(mt == MT - 1),
                )
            to = tout_p.tile([128, 512], F32, tag="to")
            if (qt_i + oc) % 2 == 0:
                nc.scalar.copy(to[:, :], pso[:, :])
            else:
                nc.vector.tensor_copy(to[:, :], pso[:, :])
            nc.sync.dma_start(out_ap[qsl, osl], to[:, :])


def build_bass(fix_waits=True):
    nc = bass.Bass("TRN2", debug=False)
    inp = {}

    def din(name, shape, dtype=F32R):
        inp[name] = nc.dram_tensor(name, list(shape), dtype, kind="ExternalInput").ap()

    din("xT", (D, S))
    din("wqT", (D, DC))
    din("wkT", (D, DC))
    din("wvT", (D, DC))
    din("bqc", (128, MT), F32)
    din("bvr", (1, DC))
    din("cosT", (128, S), F32)
    din("sinT", (128, S), F32)
    din("mdiagT", (128, 128))
    din("woT", (DC, D))
    din("ones", (1, 128))
    din("vones", (128, NST * HPC), F32R)
    inp["recs"] = nc.dram_tensor("recs", [HPC * NQB, 512], F32, kind="Internal").ap()
    inp["recr"] = nc.dram_tensor("recr", [HPC * NQB, 512], F32, kind="Internal").ap()
    out_ap = nc.dram_tensor("out", [S, D], F32, kind="ExternalOutput").ap()

    with tile.TileContext(nc) as tc:
        with ExitStack() as ctx:
            _attention_body(ctx, tc, inp, out_ap)
    if fix_waits:
        fix_engine_waits(nc)
    return nc


# ---- host-side sharding / prep ----


def make_core_inputs(x, mask, cos, sin, wq, bq, wk, wv, bv, wo):
    """Returns list of 8 input dicts (core c = batch c//4, head-group c%4)."""
    x = np.ascontiguousarray(x, dtype=np.float32)
    p = np.arange(128)
    pf = p % 64
    cosT = np.ascontiguousarray(cos.T[pf, :], dtype=np.float32)          # [128, S]
    sgn = np.where(pf < 32, -1.0, 1.0).astype(np.float32)
    sinT = np.ascontiguousarray(sgn[:, None] * sin.T[pf, :], dtype=np.float32)
    mdiagT = np.ascontiguousarray(
        (mask[0:128, 0:128].T == 0).astype(np.float32)
    )
    ones = np.ones((1, 128), dtype=np.float32)
    vones = np.ones((128, NST * HPC), dtype=np.float32)

    in_maps = []
    for c in range(8):
        b, g = c // 4, c % 4
        rows = np.arange(g * DC, (g + 1) * DC)
        vrows = rows
        bqc = np.ascontiguousarray(bq[rows].reshape(MT, 128).T, dtype=np.float32)
        in_maps.append({
            "xT": np.ascontiguousarray(x[b].T),
            "wqT": np.ascontiguousarray(wq[rows].T),
            "wkT": np.ascontiguousarray(wk[rows].T),
            "wvT": np.ascontiguousarray(wv[vrows].T),
            "bqc": bqc,
            "bvr": np.ascontiguousarray(bv[vrows][None, :]),
            "cosT": cosT,
            "sinT": sinT,
            "mdiagT": mdiagT,
            "woT": np.ascontiguousarray(wo[:, vrows].T),
            "ones": ones,
            "vones": vones,
        })
    return in_maps


_NC_CACHE = []


def kernel(x, mask, cos, sin, wq, bq, wk, wv, bv, wo, bo):
    x = np.asarray(x, dtype=np.float32)
    in_maps = make_core_inputs(
        x, np.asarray(mask), np.asarray(cos), np.asarray(sin),
        np.asarray(wq), np.asarray(bq), np.asarray(wk), np.asarray(wv),
        np.asarray(bv), np.asarray(wo),
    )
    if not _NC_CACHE:
        _NC_CACHE.append(build_bass())
    nc = _NC_CACHE[0]
    res = run_bass_kernel_spmd(nc, in_maps, core_ids=list(range(8)))
    out = np.zeros((B, S, D), dtype=np.float32)
    for c in range(8):
        out[c // 4] += res.results[c]["out"]
    out += np.asarray(bo, dtype=np.float32)[None, None, :]
    return out



# revision 4
# speedup vs baseline: 1.2231x; 1.2231x over previous
"""Multi-head causal attention (B=2,S=2048,D=1024,H=16,RoPE) on 8 TRN2 NeuronCores.

Sharding: core c handles batch b=c//4, head-group g=c%4 (4 heads each).
Wq/Wk/Wv column-sharded per head group, Wo row-sharded; the all-reduce over
head groups is realized as a host-side partial sum at gather time.

Per-core kernel, all matmul operands bf16 (fp32 PSUM accumulation):
  Projection round r: QKV projections for s-block r from pre-transposed x,
    Q/K kept feature-major [d, s], RoPE'd via partition-shift DMAs + DVE;
    V natural [s, d] with a ones column per head (softmax denominators ride
    the AV matmul).
  Attention (m, qb): per head-pair m the two heads' score matmuls are
    row-tiled (K=64 at partition bases 0/64) into one 2-bank PSUM pair tile
    and run concurrently; a single paired exp [128, 2, 512] on ACT covers
    both heads; binary diag-mask multiply on gpsimd; AV accumulation into
    [65, 512] PSUM per head.
  Weaving: projection round r+1 and output-projection round r-1 are emitted
    as fill pieces inside attention round r's kt loop so the ACT-bound
    attention stretches keep the PE busy.
  Normalize: puv evacuated to SBUF bf16 immediately (frees the PSUM bank),
    reciprocal of the denominator row on DVE, partition-broadcast via DMA on
    the scalar HWDGE ring, divide on DVE while writing vecT.
  Output projection: vecT @ Wo per 128-q chunk, bf16 partial out -> DRAM.
"""
import numpy as np
import ml_dtypes
from contextlib import ExitStack

import concourse.bass as bass
import concourse.tile as tile
from concourse import mybir
from concourse.bass_utils import run_bass_kernel_spmd

B, S, D, H, HD = 2, 2048, 1024, 16, 64
HPC = 4            # heads per core
DC = HPC * HD      # 256 features per core
NDT = D // 128     # 8 input-dim tiles
NST = S // 128     # 16 sequence/key tiles
NQB = S // 512     # 4 query blocks
MT = DC // 128     # 2 feature m-tiles for Q/K/vec

F32 = mybir.dt.float32
BF16 = mybir.dt.bfloat16
AF = mybir.ActivationFunctionType

_nop_ctr = [0]


def fix_engine_waits(nc, max_waits=1):
    """This walrus build rejects any engine instruction with >1 sync wait
    (single wait slot per instruction struct). Move excess waits onto
    same-engine NoOps inserted just before, one wait per NoOp. InstISA is
    skipped (fixed-length encoding)."""
    moved = 0
    for f in nc.m.functions:
        for b in f.blocks:
            insts = b.instructions
            i = 0
            while i < len(insts):
                inst = insts[i]
                if inst.opcode != "ISA" and inst.sync_info is not None:
                    si = inst.sync_info
                    waits = list(si.on_wait)
                    if len(waits) > max_waits:
                        keep = waits[-max_waits:]
                        for w in waits[:-max_waits]:
                            _nop_ctr[0] += 1
                            moved += 1
                            nop = mybir.InstNoOp(
                                name=f"I-waitnop-{_nop_ctr[0]}", ins=[], outs=[]
                            )
                            nop.engine = inst.engine
                            nop.sync_info = mybir.SyncInfo(on_wait=[w], on_update=[])
                            insts.insert(i, nop)
                            i += 1
                        si.on_wait = keep
                        inst.sync_info = si
                i += 1
    return moved


def _attention_body(ctx: ExitStack, tc, inp, out_ap):
    nc = tc.nc

    persist = ctx.enter_context(tc.tile_pool(name="persist", bufs=1))
    wpool = ctx.enter_context(tc.tile_pool(name="wpool", bufs=1))
    xtp = ctx.enter_context(tc.tile_pool(name="xtp", bufs=2))
    qtmp_p = ctx.enter_context(tc.tile_pool(name="qtmp", bufs=3))
    tsh_p = ctx.enter_context(tc.tile_pool(name="tsh", bufs=3))
    tb2_p = ctx.enter_context(tc.tile_pool(name="tb2", bufs=2))
    exp_p = ctx.enter_context(tc.tile_pool(name="expp", bufs=4))
    uvsb_p = ctx.enter_context(tc.tile_pool(name="uvsb", bufs=4))
    rrec_p = ctx.enter_context(tc.tile_pool(name="rrec", bufs=2))
    vtmp_p = ctx.enter_context(tc.tile_pool(name="vtmp", bufs=2))
    tout_p = ctx.enter_context(tc.tile_pool(name="toutp", bufs=2))
    ps_pair = ctx.enter_context(tc.tile_pool(name="ps_pair", bufs=2, space="PSUM"))
    ps_uvec = ctx.enter_context(tc.tile_pool(name="ps_uvec", bufs=2, space="PSUM"))
    ps_work = ctx.enter_context(tc.tile_pool(name="ps_work", bufs=2, space="PSUM"))

    # ---- persistent tensors ----
    qrt = persist.tile([128, MT, S], BF16)      # rotated Q^T  (d-major)
    krt = persist.tile([128, MT, S], BF16)      # rotated K^T
    vext = persist.tile([128, NST, HPC * 65], BF16)  # V tiles + ones col per head
    vecT = persist.tile([128, MT, S], BF16)     # normalized attention output^T
    cos_sb = persist.tile([128, S], BF16)
    sin_sb = persist.tile([128, S], BF16)
    wo_sb = persist.tile([128, MT, D], BF16)
    mdiag_sb = persist.tile([128, 128], BF16)   # binary causal mask, diag block^T
    bq_sb = persist.tile([128, MT], F32)
    bv_sb = persist.tile([1, DC], BF16)
    ones_sb = persist.tile([1, 128], BF16)

    # weights first (per d-tile so the first matmuls start early), then consts
    wq_sb = wpool.tile([128, NDT, DC], BF16)
    wk_sb = wpool.tile([128, NDT, DC], BF16)
    wv_sb = wpool.tile([128, NDT, DC], BF16)
    xT_view = inp["xT"].rearrange("(dt p) s -> p dt s", p=128)
    xts = [
        xtp.tile([128, NDT, 512], BF16, tag="xt", name=f"xt{sb}") for sb in range(2)
    ]
    for dt in range(NDT):
        nc.sync.dma_start(xts[0][:, dt, :], xT_view[:, dt, 0:512])
        for w_sb, nm in ((wq_sb, "wqT"), (wk_sb, "wkT"), (wv_sb, "wvT")):
            nc.sync.dma_start(
                w_sb[:, dt, :],
                inp[nm].rearrange("(dt p) o -> p dt o", p=128)[:, dt, :],
            )
    nc.scalar.dma_start(cos_sb[:, :], inp["cosT"])
    nc.scalar.dma_start(sin_sb[:, :], inp["sinT"])
    nc.scalar.dma_start(mdiag_sb[:, :], inp["mdiagT"])
    nc.scalar.dma_start(bq_sb[:, :], inp["bqc"])
    nc.scalar.dma_start(bv_sb[:, :], inp["bvr"])
    nc.scalar.dma_start(ones_sb[:, :], inp["ones"])
    nc.sync.dma_start(
        wo_sb[:, :, :], inp["woT"].rearrange("(mt p) o -> p mt o", p=128)
    )
    # ones columns of vext (col 64 of each head slot, every k-tile)
    vones_dst = vext[:, :, :].rearrange("p st (h e) -> p st h e", e=65)[:, :, :, 64:65]
    nc.scalar.dma_start(
        vones_dst, inp["vones"].rearrange("p (st h e) -> p st h e", st=NST, h=HPC)
    )
    for dt in range(NDT):
        nc.sync.dma_start(xts[1][:, dt, :], xT_view[:, dt, 512:1024])

    def prefetch_xt(sb):
        xtn = xtp.tile([128, NDT, 512], BF16, tag="xt", name=f"xt{sb}")
        for dt in range(NDT):
            nc.sync.dma_start(xtn[:, dt, :], xT_view[:, dt, sb * 512 : (sb + 1) * 512])
        return xtn

    # ---- phase pieces ----
    def qk_proj_piece(m, sb, which, xt):
        def run():
            ssl = slice(sb * 512, (sb + 1) * 512)
            is_q = which == 0
            dst = qrt if is_q else krt
            w_sb = wq_sb if is_q else wk_sb
            tag_q = "q" if is_q else "k"
            psq = ps_work.tile([128, 512], F32, tag="w", name=f"psq{tag_q}_{m}_{sb}")
            for dt in range(NDT):
                nc.tensor.matmul(
                    psq[:, :],
                    w_sb[:, dt, m * 128 : (m + 1) * 128],
                    xt[:, dt, :],
                    start=(dt == 0),
                    stop=(dt == NDT - 1),
                )
            qt = qtmp_p.tile([128, 512], BF16, tag="qt", name=f"qt{tag_q}_{m}_{sb}")
            if is_q:
                nc.scalar.activation(
                    qt[:, :], psq[:, :], AF.Identity, bias=bq_sb[:, m : m + 1]
                )
            else:
                nc.scalar.copy(qt[:, :], psq[:, :])
            # rotate_half partition shift p ^ 32 via 4 contiguous DMAs
            sh = tsh_p.tile([128, 512], BF16, tag="sh", name=f"sh{tag_q}_{m}_{sb}")
            for base in (0, 64):
                nc.scalar.dma_start(
                    sh[base : base + 32, :], qt[base + 32 : base + 64, :]
                )
                nc.scalar.dma_start(
                    sh[base + 32 : base + 64, :], qt[base : base + 32, :]
                )
            dsl = dst[:, m, ssl]
            tb2 = tb2_p.tile([128, 512], BF16, tag="tb2", name=f"tb2{tag_q}_{m}_{sb}")
            nc.vector.tensor_mul(dsl, qt[:, :], cos_sb[:, ssl])
            nc.vector.tensor_mul(tb2[:, :], sh[:, :], sin_sb[:, ssl])
            nc.vector.tensor_add(dsl, dsl, tb2[:, :])
        return run

    def v_proj_piece(st, xt):
        def run():
            psv = ps_work.tile([128, 512], F32, tag="w", name=f"psv_{st}")[:, 0:256]
            for dt in range(NDT):
                nc.tensor.matmul(
                    psv[:, :],
                    xt[:, dt, (st % 4) * 128 : (st % 4 + 1) * 128],
                    wv_sb[:, dt, :],
                    start=(dt == 0),
                    stop=False,
                )
            nc.tensor.matmul(
                psv[:, :], ones_sb[0:1, :], bv_sb[0:1, :], start=False, stop=True
            )
            vdst = vext[:, st, :].rearrange("p (h e) -> p h e", e=65)[:, :, 0:64]
            nc.vector.tensor_copy(vdst, psv[:, :].rearrange("p (h e) -> p h e", e=64))
        return run

    def outproj_piece(qt_i, oc):
        def run():
            qsl = slice(qt_i * 128, (qt_i + 1) * 128)
            osl = slice(oc * 512, (oc + 1) * 512)
            pso = ps_work.tile([128, 512], F32, tag="w", name=f"pso_{qt_i}_{oc}")
            for mt in range(MT):
                nc.tensor.matmul(
                    pso[:, :],
                    vecT[:, mt, qsl],
                    wo_sb[:, mt, osl],
                    start=(mt == 0),
                    stop=(mt == MT - 1),
                )
            to = tout_p.tile([128, 512], BF16, tag="to", name=f"to_{qt_i}_{oc}")
            nc.vector.tensor_copy(to[:, :], pso[:, :])
            nc.sync.dma_start(out_ap[qsl, osl], to[:, :])
        return run

    def normalize(m, hp, qb, puv_t):
        qsl = slice(qb * 512, (qb + 1) * 512)
        h = 2 * m + hp
        # evacuate PSUM immediately so the bank frees fast
        uv = uvsb_p.tile([65, 512], BF16, tag="uv", name=f"uv_{h}_{qb}")
        nc.vector.tensor_copy(uv[:, :], puv_t[:, :])
        r_sb = rrec_p.tile([1, 512], F32, tag="rs", name=f"rsb_{h}_{qb}")
        nc.vector.reciprocal(r_sb[:, :], uv[64:65, :])
        ridx = h * NQB + qb
        nc.scalar.dma_start(inp["recr"][ridx : ridx + 1, :], r_sb[0:1, :])
        rb = rrec_p.tile([64, 512], F32, tag="rb", name=f"rb_{h}_{qb}")
        nc.scalar.dma_start(
            rb[:, :], inp["recr"][ridx : ridx + 1, :].to_broadcast([64, 512])
        )
        if hp == 0:
            nc.vector.tensor_mul(vecT[0:64, m, qsl], uv[0:64, :], rb[:, :])
        else:
            vt = vtmp_p.tile([64, 512], BF16, tag="vt", name=f"vt_{h}_{qb}")
            nc.vector.tensor_mul(vt[:, :], uv[0:64, :], rb[:, :])
            nc.scalar.dma_start(vecT[64:128, m, qsl], vt[:, :])

    # ---- attention with woven fill pieces ----
    fill_queue = []

    def emit_fill(n):
        for _ in range(n):
            if fill_queue:
                fill_queue.pop(0)()

    def attn_qb(m, qb, fill_every):
        puv = [
            ps_uvec.tile([65, 512], F32, tag="u", name=f"puv_m{m}h{hp}q{qb}")
            for hp in range(2)
        ]
        nkt = 4 * qb + 4
        pending = None
        for kt in range(nkt + 1):
            if kt < nkt:
                qb0 = kt // 4
                c0 = (kt % 4) * 128 if qb == qb0 else 0
                psc = ps_pair.tile(
                    [128, 2, 512], F32, tag="pair", name=f"psc_m{m}q{qb}k{kt}"
                )
                for hp in range(2):
                    pb = hp * 64
                    nc.tensor.matmul(
                        psc[:, hp, c0:512],
                        krt[pb : pb + 64, m, kt * 128 : (kt + 1) * 128],
                        qrt[pb : pb + 64, m, qb * 512 + c0 : (qb + 1) * 512],
                        start=True,
                        stop=True,
                    )
                et = exp_p.tile(
                    [128, 2, 512], BF16, tag="e", name=f"et_m{m}q{qb}k{kt}"
                )
                nc.scalar.activation(
                    et[:, :, c0:512], psc[:, :, c0:512], AF.Exp, scale=0.125
                )
                if qb == qb0:
                    for hp in range(2):
                        nc.gpsimd.tensor_mul(
                            et[:, hp, c0 : c0 + 128],
                            et[:, hp, c0 : c0 + 128],
                            mdiag_sb[:, :],
                        )
                cur = (kt, c0, et)
            else:
                cur = None
            if pending is not None:
                pkt, pc0, pet = pending
                for hp in range(2):
                    h = 2 * m + hp
                    nc.tensor.matmul(
                        puv[hp][:, pc0:512],
                        vext[:, pkt, h * 65 : (h + 1) * 65],
                        pet[:, hp, pc0:512],
                        start=(pkt == 0),
                        stop=(pkt == nkt - 1),
                        skip_group_check=True,
                    )
            pending = cur
            if fill_every and (kt % fill_every == fill_every - 1):
                emit_fill(1)
        for hp in range(2):
            normalize(m, hp, qb, puv[hp])

    # round 0 projections standalone
    for m in range(MT):
        qk_proj_piece(m, 0, 0, xts[0])()
    v_proj_piece(0, xts[0])()
    v_proj_piece(1, xts[0])()
    for m in range(MT):
        qk_proj_piece(m, 0, 1, xts[0])()
    v_proj_piece(2, xts[0])()
    v_proj_piece(3, xts[0])()

    for r in range(NQB):
        if r + 2 < NQB:
            xts.append(prefetch_xt(r + 2))
        pieces = []
        if r + 1 < NQB:
            xt_n = xts[r + 1]
            pieces += [
                qk_proj_piece(0, r + 1, 0, xt_n),
                qk_proj_piece(1, r + 1, 0, xt_n),
                v_proj_piece(4 * (r + 1) + 0, xt_n),
                v_proj_piece(4 * (r + 1) + 1, xt_n),
                qk_proj_piece(0, r + 1, 1, xt_n),
                qk_proj_piece(1, r + 1, 1, xt_n),
                v_proj_piece(4 * (r + 1) + 2, xt_n),
                v_proj_piece(4 * (r + 1) + 3, xt_n),
            ]
        if r > 0:
            outp = [
                outproj_piece(4 * (r - 1) + i, oc) for i in range(4) for oc in range(2)
            ]
            # interleave the small outproj pieces between the big proj pieces
            mixed = []
            while pieces or outp:
                if pieces:
                    mixed.append(pieces.pop(0))
                if outp:
                    mixed.append(outp.pop(0))
            pieces = mixed
        fill_queue.extend(pieces)
        nkts = 2 * (4 * r + 4)
        fill_every = max(1, nkts // (len(fill_queue) + 1)) if fill_queue else 0
        attn_qb(0, r, fill_every)
        attn_qb(1, r, fill_every)
        emit_fill(len(fill_queue))

    for i in range(4):
        for oc in range(2):
            outproj_piece(12 + i, oc)()


def build_bass(fix_waits=True):
    nc = bass.Bass("TRN2", debug=False)
    inp = {}

    def din(name, shape, dtype=BF16):
        inp[name] = nc.dram_tensor(name, list(shape), dtype, kind="ExternalInput").ap()

    din("xT", (D, S))
    din("wqT", (D, DC))
    din("wkT", (D, DC))
    din("wvT", (D, DC))
    din("bqc", (128, MT), F32)
    din("bvr", (1, DC))
    din("cosT", (128, S))
    din("sinT", (128, S))
    din("mdiagT", (128, 128))
    din("woT", (DC, D))
    din("ones", (1, 128))
    din("vones", (128, NST * HPC))
    inp["recr"] = nc.dram_tensor(
        "recr", [HPC * NQB, 512], F32, kind="Internal"
    ).ap()
    out_ap = nc.dram_tensor("out", [S, D], BF16, kind="ExternalOutput").ap()

    with tile.TileContext(nc) as tc:
        with ExitStack() as ctx:
            _attention_body(ctx, tc, inp, out_ap)
    if fix_waits:
        fix_engine_waits(nc)
    return nc


# ---- host-side sharding / prep ----


def make_core_inputs(x, mask, cos, sin, wq, bq, wk, wv, bv, wo):
    """Returns list of 8 input dicts (core c = batch c//4, head-group c%4)."""
    bf16 = ml_dtypes.bfloat16
    x = np.ascontiguousarray(x, dtype=np.float32)
    p = np.arange(128)
    pf = p % 64
    cosT = np.ascontiguousarray(cos.T[pf, :]).astype(bf16)          # [128, S]
    sgn = np.where(pf < 32, -1.0, 1.0).astype(np.float32)
    sinT = np.ascontiguousarray(sgn[:, None] * sin.T[pf, :]).astype(bf16)
    mdiagT = np.ascontiguousarray(
        (mask[0:128, 0:128].T == 0).astype(np.float32)
    ).astype(bf16)
    ones = np.ones((1, 128), dtype=bf16)
    vones = np.ones((128, NST * HPC), dtype=bf16)

    in_maps = []
    for c in range(8):
        b, g = c // 4, c % 4
        rows = np.arange(g * DC, (g + 1) * DC)
        bqc = np.ascontiguousarray(bq[rows].reshape(MT, 128).T, dtype=np.float32)
        in_maps.append({
            "xT": np.ascontiguousarray(x[b].T).astype(bf16),
            "wqT": np.ascontiguousarray(wq[rows].T).astype(bf16),
            "wkT": np.ascontiguousarray(wk[rows].T).astype(bf16),
            "wvT": np.ascontiguousarray(wv[rows].T).astype(bf16),
            "bqc": bqc,
            "bvr": np.ascontiguousarray(bv[rows][None, :]).astype(bf16),
            "cosT": cosT,
            "sinT": sinT,
            "mdiagT": mdiagT,
            "woT": np.ascontiguousarray(wo[:, rows].T).astype(bf16),
            "ones": ones,
            "vones": vones,
        })
    return in_maps


_NC_CACHE = []


def kernel(x, mask, cos, sin, wq, bq, wk, wv, bv, wo, bo):
    x = np.asarray(x, dtype=np.float32)
    in_maps = make_core_inputs(
        x, np.asarray(mask), np.asarray(cos), np.asarray(sin),
        np.asarray(wq), np.asarray(bq), np.asarray(wk), np.asarray(wv),
        np.asarray(bv), np.asarray(wo),
    )
    if not _NC_CACHE:
        _NC_CACHE.append(build_bass())
    nc = _NC_CACHE[0]
    res = run_bass_kernel_spmd(nc, in_maps, core_ids=list(range(8)))
    out = np.zeros((B, S, D), dtype=np.float32)
    for c in range(8):
        out[c // 4] += np.asarray(res.results[c]["out"], dtype=np.float32)
    out += np.asarray(bo, dtype=np.float32)[None, None, :]
    return out
